# revision 14
# baseline (speedup 1.0000x reference)
"""Trainium2 Bass kernel for nn_LMAttention_25262997635622 (v2, bf16).

Prefill GQA attention layer: B=1, T=1024, DIM=3072, H=32 q-heads,
KVH=8 kv-heads, D=128 head dim, interleaved-pair RoPE, causal mask.
input_pos = arange(T) and the caches arrive zeroed, so attention reduces
to causal self-attention over the freshly projected K/V.

Sharding (8 cores, tensor-parallel over heads):
  core p: q-heads [4p, 4p+4), kv-head p.
  wq/wk/wv sharded on output dim, wo sharded on input dim; x replicated.
  Each core computes a partial (DIM, T) output; the host sums the 8
  partials (fp32) as the unshard step.

v2 design notes (vs the fp32r baseline):
  - All matmul operands are bf16 (fp32 PSUM accumulation). This halves
    HBM traffic and turns on fast-weight-load so LDWEIGHTS hides under
    the matmul stream (fp32r blocks FWL; LDW was the PE rate limiter).
  - Chunk-major pipeline: for each 512-token chunk j, run projections ->
    RoPE -> attention (all 4 heads) -> wo partial + output DMA. Output
    DMA streams from mid-kernel instead of pooling in a 45us tail, and
    the PE never sees a multi-us phase boundary (HAM stays warm).
  - DMA triggers are emitted in exact consumption order with per-group
    weight pieces so the first matmul is gated on ~4MB, not 27MB.
  - Score/exp/PV/sum instructions are column-trimmed at the causal
    diagonal (N = 512-128*rr) instead of compute-then-memset.
  - RoPE uses stacked cos/sin tables ([c;c], [s;s]) so all 4 ops run on
    full 128 partitions: 2 PSUM-read multiplies + 2 bf16 cross-half
    combines per (head, chunk).
  - Softmax normalization: ones-column matmul accumulates the exp row
    sums; reciprocal_approx_fast (not the 4us iterative reciprocal);
    ones-row matmul broadcasts it across partitions; one DVE multiply.
    Each head's normalization is deferred behind the next head's first
    score/exp so it never stalls the PE.
  - Causal tri-masks run on GpSimd (otherwise idle) so they don't queue
    behind RoPE work on the DVE.
"""

import math
import sys
from contextlib import ExitStack

import numpy as np

sys.path.insert(0, "/opt/trn_rl_repo")

import ml_dtypes

import concourse.bass as bass
import concourse.mybir as mybir
import concourse.tile as tile
from concourse import bacc
from concourse.bass_utils import run_bass_kernel_spmd

B, T, DIM = 1, 1024, 3072
H, KVH, D = 32, 8, 128
NCORES = 8
HQ = H // NCORES          # q-heads per core = 4
E = HQ * D                # q features per core = 512
P = 128                   # partitions
KO = DIM // P             # k-tiles over DIM = 24
NG = 4                    # weight DMA groups
KG = KO // NG             # ko per weight DMA group = 6
TQC = 512                 # t chunk (one fp32 PSUM bank)
NCH = T // TQC            # 2 chunks
NKB = T // P              # t_k blocks = 8
SCALE = 1.0 / math.sqrt(D)

F32 = mybir.dt.float32
F32R = mybir.dt.float32r
BF16 = mybir.dt.bfloat16
U16 = mybir.dt.uint16
MUL = mybir.AluOpType.mult
SUB = mybir.AluOpType.subtract
ADD = mybir.AluOpType.add
EXP = mybir.ActivationFunctionType.Exp
BF = ml_dtypes.bfloat16


def build_kernel():
    nc = bacc.Bacc(None, target_bir_lowering=False)

    x_d = nc.declare_dram_parameter("x2", [NCH, P, KO, TQC], BF16, isOutput=False)
    wq_d = nc.declare_dram_parameter("wq4", [NG, P, KG, E], BF16, isOutput=False)
    wk_d = nc.declare_dram_parameter("wk4", [NG, P, KG, D], BF16, isOutput=False)
    wv_d = nc.declare_dram_parameter("wv4", [NG, P, KG, D], BF16, isOutput=False)
    wo_d = nc.declare_dram_parameter("wo2", [2, P, HQ, DIM // 2], BF16, isOutput=False)
    cos2_d = nc.declare_dram_parameter("cos2", [P, T], F32, isOutput=False)
    sin2_d = nc.declare_dram_parameter("sin2", [P, T], F32, isOutput=False)
    # tri[p, c] = 1 if p <= c (causal keep-mask for a diagonal 128 block)
    tri_d = nc.declare_dram_parameter("tri", [P, P], BF16, isOutput=False)
    iden_d = nc.declare_dram_parameter("iden", [P, P], BF16, isOutput=False)
    y_d = nc.declare_dram_parameter("yT", [NCH, KO, P, TQC], BF16, isOutput=True)
    y3 = y_d.ap()

    with tile.TileContext(nc) as tc, ExitStack() as ctx:
        const = ctx.enter_context(tc.tile_pool(name="const", bufs=1))
        pers = ctx.enter_context(tc.tile_pool(name="pers", bufs=1))
        ppool = ctx.enter_context(tc.tile_pool(name="ppool", bufs=4))
        npool = ctx.enter_context(tc.tile_pool(name="npool", bufs=2))
        opool = ctx.enter_context(tc.tile_pool(name="opool", bufs=3))
        psum = ctx.enter_context(tc.tile_pool(name="psum", bufs=8, space="PSUM"))

        def pstile(shape=None, dtype=F32):
            return psum.tile(shape or [P, TQC], dtype, name="ps", tag="ps")

        # ---- persistent SBUF tensors ----
        xsb = pers.tile([P, NCH, KO, TQC], BF16)   # full x, chunk-major
        wq_sb = pers.tile([P, KO, E], BF16)
        wk_sb = pers.tile([P, KO, D], BF16)
        wv_sb = pers.tile([P, KO, D], BF16)
        wo_sb = pers.tile([P, HQ, DIM], BF16)
        cos2 = const.tile([P, T], F32)             # [cos; cos] stacked
        sin2 = const.tile([P, T], F32)
        tri = const.tile([P, P], BF16)
        iden = const.tile([P, P], BF16)
        ones_col = const.tile([P, 1], BF16)
        ones_row = const.tile([1, P], BF16)
        qT = pers.tile([P, HQ, T], BF16)           # [dhead, q-head, t]
        kT = pers.tile([P, T], BF16)               # [dhead, t]
        v = pers.tile([P, NKB, D], BF16)           # [t_k in block, block, dv]
        attnT = pers.tile([P, HQ, T], BF16)        # normalized PV, [dv, head, t]

        # ---- DMA triggers in consumption order (single sync queue) ----
        # x chunk 0 in ko-halves so the k-projection starts after ~1.5MB;
        # wk (consumed first) ahead of wv/wq within each group.
        nc.sync.dma_start(xsb[:, 0, bass.ts(0, KO // 2)],
                          x_d.ap()[0][:, bass.ts(0, KO // 2)])
        for g in range(NG // 2):
            nc.sync.dma_start(wk_sb[:, bass.ts(g, KG)], wk_d.ap()[g])
        nc.sync.dma_start(xsb[:, 0, bass.ts(1, KO // 2)],
                          x_d.ap()[0][:, bass.ts(1, KO // 2)])
        for g in range(NG // 2, NG):
            nc.sync.dma_start(wk_sb[:, bass.ts(g, KG)], wk_d.ap()[g])
        for g in range(NG):
            nc.sync.dma_start(wv_sb[:, bass.ts(g, KG)], wv_d.ap()[g])
        for g in range(NG):
            nc.sync.dma_start(wq_sb[:, bass.ts(g, KG)], wq_d.ap()[g])
        nc.sync.dma_start(cos2[:], cos2_d.ap())
        nc.sync.dma_start(sin2[:], sin2_d.ap())
        nc.sync.dma_start(tri[:], tri_d.ap())
        nc.sync.dma_start(iden[:], iden_d.ap())
        nc.sync.dma_start(xsb[:, 1], x_d.ap()[1])
        nc.sync.dma_start(wo_sb[:, :, bass.ts(0, DIM // 2)], wo_d.ap()[0])
        nc.sync.dma_start(wo_sb[:, :, bass.ts(1, DIM // 2)], wo_d.ap()[1])
        nc.vector.memset(ones_col[:].bitcast(U16), 0x3F80)  # bf16 1.0
        nc.vector.memset(ones_row[:].bitcast(U16), 0x3F80)

        # PE warmup on scratch zeros while the input DMA streams: sustained
        # matmul activity releases the HAM clock throttle (~3.4us window) so
        # the first real matmuls run at 2.4GHz instead of 1.2GHz, and the
        # activity bridge prevents an early re-throttle.
        warm = const.tile([P, TQC], BF16)
        nc.vector.memset(warm[:].bitcast(U16), 0)
        wps = pstile()
        for _ in range(32):
            nc.tensor.matmul(wps[:], warm[:, bass.ts(0, P)], warm[:],
                             start=True, stop=True)

        def _rope2(ps, j, out):
            """out[:64] = r*c - i*s ; out[64:] = r*s + i*c  (bf16 out).

            ps: [128, TQC] fp32 PSUM with de-interleaved rows [r(64); i(64)].
            Both inputs of every tensor_tensor share a base partition
            (walrus NCC_IBIR297); only outputs land at an offset base.
            """
            h = D // 2
            cs = cos2[:, bass.ts(j, TQC)]
            sn = sin2[:, bass.ts(j, TQC)]
            m1 = ppool.tile([P, TQC], BF16, name="m1", tag="rope")
            m2 = ppool.tile([P, TQC], BF16, name="m2", tag="rope")
            nc.vector.tensor_tensor(m1[:], ps[:], cs, MUL)            # [r*c; i*c]
            nc.vector.tensor_tensor(m2[:h], ps[h:], sn[h:], MUL)      # i*s
            nc.vector.tensor_tensor(m2[h:], ps[:h], sn[:h], MUL)      # r*s
            nc.vector.tensor_tensor(out[:h], m1[:h], m2[:h], SUB)     # r*c - i*s
            nc.vector.tensor_tensor(out[h:], m1[h:], m2[h:], ADD)     # i*c + r*s

        # ---- phase generators: each yield = one PE work item, so two
        # phases can be interleaved instruction-by-instruction (the PE
        # executes in order; interleaving fills one phase's dependency
        # stalls with the other phase's ready matmuls) ----

        def proj_gen(j):
            """Projections + RoPE + v-transpose for chunk j (148 items)."""
            tq = bass.ts(j, TQC)
            # k
            psk = pstile()
            for ko in range(KO):
                nc.tensor.matmul(psk[:], wk_sb[:, ko], xsb[:, j, ko],
                                 start=ko == 0, stop=ko == KO - 1)
                yield
            _rope2(psk, j, kT[:, tq])
            # vT
            psvt = pstile()
            for ko in range(KO):
                nc.tensor.matmul(psvt[:], wv_sb[:, ko], xsb[:, j, ko],
                                 start=ko == 0, stop=ko == KO - 1)
                yield
            vt_sb = ppool.tile([P, TQC], BF16, name="vt", tag="vt")
            nc.scalar.copy(vt_sb[:], psvt[:])
            # q0..q3
            for mi in range(HQ):
                psq = pstile()
                for ko in range(KO):
                    nc.tensor.matmul(psq[:], wq_sb[:, ko, bass.ts(mi, P)],
                                     xsb[:, j, ko],
                                     start=ko == 0, stop=ko == KO - 1)
                    yield
                _rope2(psq, j, qT[:, mi, tq])
            # vT [dv, t] -> v [t, dv] via PE transpose per 128 block
            for b in range(TQC // P):
                pst = pstile([P, P], BF16)
                nc.tensor.transpose(pst[:], vt_sb[:, bass.ts(b, P)], iden[:])
                nc.vector.tensor_copy(out=v[:, 4 * j + b], in_=pst[:])
                yield

        def att_gen(j):
            """Attention for chunk j, heads sequential (4*(nvis+1) items)."""
            nvis = 4 * (j + 1)  # visible t_k blocks
            tq = bass.ts(j, TQC)

            def sc_exp(m, i):
                """Scores + exp (+ diagonal mask) for (head m, block i)."""
                rr = i - 4 * j
                n0 = rr * P if rr > 0 else 0
                w = TQC - n0
                s_ps = pstile()
                nc.tensor.matmul(
                    s_ps[:, bass.ds(n0, w)], kT[:, bass.ts(i, P)],
                    qT[:, m, bass.ds(j * TQC + n0, w)], start=True, stop=True,
                )
                pt = ppool.tile([P, TQC], BF16, name="pt", tag="pt")
                nc.scalar.activation(pt[:, bass.ds(n0, w)],
                                     s_ps[:, bass.ds(n0, w)], EXP, scale=SCALE)
                if rr >= 0:
                    # early heads mask on GpSimd: the DVE may still be
                    # draining this chunk's RoPE backlog
                    eng = nc.gpsimd if m < 2 else nc.vector
                    eng.tensor_tensor(pt[:, bass.ds(n0, P)],
                                      pt[:, bass.ds(n0, P)], tri[:], MUL)
                return pt, n0

            prev_norm = None
            for m in range(HQ):
                att = pstile()
                smp = pstile([1, TQC])
                pts = {0: sc_exp(m, 0)}
                yield
                for i in range(nvis):
                    if i + 1 < nvis:
                        pts[i + 1] = sc_exp(m, i + 1)
                    pt, n0 = pts.pop(i)
                    w = TQC - n0
                    st, sp = i == 0, i == nvis - 1
                    nc.tensor.matmul(att[:, bass.ds(n0, w)], v[:, i],
                                     pt[:, bass.ds(n0, w)], start=st, stop=sp)
                    nc.tensor.matmul(smp[:, bass.ds(n0, w)], ones_col[:],
                                     pt[:, bass.ds(n0, w)], start=st, stop=sp)
                    if i == 0 and prev_norm is not None:
                        prev_norm()  # previous head's PE-side normalization
                    yield

                # exp row-sums leave PSUM (DVE copy, bf16, keeps ACT free
                # for exps); the PE-side broadcast + reciprocal + multiply
                # defer into the next head
                rec = npool.tile([1, TQC], BF16, name="rec", tag="rec")
                nc.vector.tensor_copy(out=rec[:], in_=smp[:])

                def make_norm(att=att, rec=rec, m=m):
                    def norm():
                        bc = pstile()
                        nc.tensor.matmul(bc[:], ones_row[:], rec[:],
                                         start=True, stop=True)
                        rec_sb = npool.tile([P, TQC], F32, name="rbc", tag="rbc")
                        nc.vector.reciprocal_approx_fast(rec_sb[:], bc[:])
                        nc.vector.tensor_tensor(attnT[:, m, tq], att[:],
                                                rec_sb[:], MUL)
                    return norm

                prev_norm = make_norm()
            prev_norm()

        def wo_gen(j):
            """Output projection for chunk j (24 items)."""
            tq = bass.ts(j, TQC)
            for mo in range(KO):
                ps_y = pstile()
                for eo in range(HQ):
                    nc.tensor.matmul(ps_y[:], wo_sb[:, eo, bass.ts(mo, P)],
                                     attnT[:, eo, tq],
                                     start=eo == 0, stop=eo == HQ - 1)
                ysb = opool.tile([P, TQC], BF16, name="ysb", tag="ysb")
                if mo % 2:
                    nc.vector.tensor_copy(out=ysb[:], in_=ps_y[:])
                else:
                    nc.scalar.copy(ysb[:], ps_y[:])
                nc.sync.dma_start(y3[j, mo], ysb[:])
                yield

        def interleave(primary, filler, fill_per_item):
            """One primary item, then ~fill_per_item filler items, repeat."""
            acc = 0.0
            exhausted = False
            for _ in primary:
                acc += fill_per_item
                while acc >= 1.0 and not exhausted:
                    try:
                        next(filler)
                    except StopIteration:
                        exhausted = True
                    acc -= 1.0
            if not exhausted:
                for _ in filler:
                    pass

        # proj(0) | att(0) x proj(1) | att(1) x wo(0) | wo(1)
        for _ in proj_gen(0):
            pass
        interleave(att_gen(0), proj_gen(1), 148 / 20)
        interleave(att_gen(1), wo_gen(0), 24 / 36)
        for _ in wo_gen(1):
            pass

    nc.compile()
    return nc


_NC_CACHE = None


def _get_nc():
    global _NC_CACHE
    if _NC_CACHE is None:
        _NC_CACHE = build_kernel()
    return _NC_CACHE


def _prep_in_maps(inputs):
    x = np.asarray(inputs["x"], np.float32)[0]       # (T, DIM)
    wq = np.asarray(inputs["wq"], np.float32)        # (H*D, DIM)
    wk = np.asarray(inputs["wk"], np.float32)        # (KVH*D, DIM)
    wv = np.asarray(inputs["wv"], np.float32)        # (KVH*D, DIM)
    wo = np.asarray(inputs["wo"], np.float32)        # (DIM, H*D)
    fc = np.asarray(inputs["freqs_cos"], np.float32)  # (T, D//2)
    fs = np.asarray(inputs["freqs_sin"], np.float32)

    # de-interleave permutation within each head
    perm = np.concatenate([np.arange(0, D, 2), np.arange(1, D, 2)])

    xT = x.T                                         # (DIM, T)
    x2 = np.ascontiguousarray(
        xT.reshape(KO, P, NCH, TQC).transpose(2, 1, 0, 3)).astype(BF)
    cos2 = np.ascontiguousarray(np.concatenate([fc.T, fc.T], axis=0))
    sin2 = np.ascontiguousarray(np.concatenate([fs.T, fs.T], axis=0))
    tri = (np.arange(P)[:, None] <= np.arange(P)[None, :]).astype(BF)
    iden = np.eye(P, dtype=np.float32).astype(BF)

    wq_h = wq.reshape(H, D, DIM)[:, perm, :]
    wk_h = wk.reshape(KVH, D, DIM)[:, perm, :]

    def wsplit(wT, n):
        # (DIM, n) -> [NG, P, KG, n] in ko-group pieces
        return np.ascontiguousarray(
            wT.reshape(NG, KG, P, n).transpose(0, 2, 1, 3)).astype(BF)

    in_maps = []
    for c in range(NCORES):
        wq_c = wq_h[HQ * c: HQ * (c + 1)].reshape(E, DIM)
        wk_c = wk_h[c]
        wv_c = wv.reshape(KVH, D, DIM)[c]
        woT_c = wo[:, E * c: E * (c + 1)].T          # (E, DIM)
        wo2 = np.ascontiguousarray(
            woT_c.reshape(HQ, P, 2, DIM // 2).transpose(2, 1, 0, 3)).astype(BF)
        in_maps.append({
            "x2": x2,
            "wq4": wsplit(wq_c.T, E),
            "wk4": wsplit(wk_c.T, D),
            "wv4": wsplit(wv_c.T, D),
            "wo2": wo2,
            "cos2": cos2,
            "sin2": sin2,
            "tri": tri,
            "iden": iden,
        })
    return in_maps


def _unshard(results):
    acc = np.zeros((DIM, T), np.float32)
    for rmap in results:
        y = np.asarray(rmap["yT"]).astype(np.float32)  # [NCH, KO, P, TQC]
        acc += y.transpose(1, 2, 0, 3).reshape(DIM, T)
    return np.ascontiguousarray(acc.T)[None]


def kernel(**inputs) -> np.ndarray:
    in_maps = _prep_in_maps(inputs)
    nc = _get_nc()
    res = run_bass_kernel_spmd(nc, in_maps, core_ids=list(range(NCORES)))
    return _unshard(res.results)


if __name__ == "__main__":
    rng = np.random.default_rng(0)
    ins = {
        "x": rng.standard_normal((1, T, DIM), dtype=np.float32),
        "wq": (rng.standard_normal((H * D, DIM)) * 0.02).astype(np.float32),
        "wk": (rng.standard_normal((KVH * D, DIM)) * 0.02).astype(np.float32),
        "wv": (rng.standard_normal((KVH * D, DIM)) * 0.02).astype(np.float32),
        "wo": (rng.standard_normal((DIM, H * D)) * 0.02).astype(np.float32),
        "freqs_cos": rng.random((T, D // 2), dtype=np.float32),
        "freqs_sin": rng.random((T, D // 2), dtype=np.float32),
        "k_cache": np.zeros((1, 4096, KVH, D), np.float32),
        "v_cache": np.zeros((1, 4096, KVH, D), np.float32),
        "input_pos": np.arange(T, dtype=np.int32),
    }
    out = kernel(**ins)
    print(out.shape, out.dtype)
